# revision 21
# baseline (speedup 1.0000x reference)
"""Distributed Trainium2 kernel for a pre-LN single attention block.

Reference computation (dims hardcoded):
    x: [4, 2048, 1024]; LN(x) -> q = xn@Wq, kv = xn@Wkv; 16 heads x 64;
    softmax(q k^T / 8) v ; out proj [1024,1024] + bias.

Sharding over 8 NeuronCores: core c handles batch b = c//2 and head
group g = c%2 (8 heads each).  Each core computes LN(x[b]) (duplicated
across the pair, cheap), its 512-wide q/k/v projection slices, its 8
attention heads and a PARTIAL out-projection (contraction over its 512
inner columns).  The two partials per batch are summed on the host
during unshard -- no on-chip collectives.  gamma is folded into the
projection weights on the host; bout is fed only to g==0 cores (zeros
to g==1) so the SPMD graph is identical on all cores.

All matmuls run in bf16 with f32 PSUM accumulation.  Attention scores
are built transposed (scoresT[j, i]) so softmax's exp lands in the
layout attn@v needs; the row-sum for softmax comes from an appended
ones-column in v; the max-subtraction is skipped (scores ~ N(0,1) after
LN, |s| < ~5, exp is safe).  The query/sequence axis of attention is
processed in two halves to halve the probability-tile SBUF footprint.
"""

import numpy as np
from contextlib import ExitStack

import concourse.bass as bass
import concourse.bacc as bacc_mod
import concourse.mybir as mybir
import concourse.tile as tile
from concourse.bass_utils import run_bass_kernel_spmd
from concourse.masks import make_identity

F32 = mybir.dt.float32
BF16 = mybir.dt.bfloat16
AF = mybir.ActivationFunctionType

B = 4
N = 2048          # sequence length
D = 1024          # model dim
GC = 512          # per-core inner columns (8 heads x 64)
DH = 64           # head dim
HPC = 8           # heads per core
P = 128
NT_I = N // P     # 16 sequence tiles
NT_C = D // P     # 8 model-dim tiles
NT_G = GC // P    # 4 inner tiles
IH = N // 2       # attention i-half length (1024)
SCALE = DH ** -0.5
EPS = 1e-5

LAST_EXEC_NS = None
LAST_TRACE = None
_CACHED_NC = None


def build_nc():
    nc = bacc_mod.Bacc()
    x_d = nc.declare_dram_parameter("x", [N, D], F32, isOutput=False)
    wq_d = nc.declare_dram_parameter("wq", [D, GC], F32, isOutput=False)
    wk_d = nc.declare_dram_parameter("wk", [D, GC], F32, isOutput=False)
    wv_d = nc.declare_dram_parameter("wv", [D, GC], F32, isOutput=False)
    wo_d = nc.declare_dram_parameter("wout", [GC, D], F32, isOutput=False)
    bo_d = nc.declare_dram_parameter("bout", [1, D], F32, isOutput=False)
    out_d = nc.declare_dram_parameter("out", [N, D], F32, isOutput=True)
    zs_d = nc.dram_tensor("zscratch", [HPC * 2, IH], F32)

    ctx = ExitStack()
    with ctx:
        tc = ctx.enter_context(tile.TileContext(nc))

        # outer pools, live for the whole kernel
        const = ctx.enter_context(tc.tile_pool(name="const", bufs=1))
        wpool = ctx.enter_context(tc.tile_pool(name="wpool", bufs=1))
        small = ctx.enter_context(tc.tile_pool(name="small", bufs=4))
        ao_pool = ctx.enter_context(tc.tile_pool(name="aoT", bufs=1))

        identity = const.tile([P, P], BF16, tag="identity")
        make_identity(nc, identity)
        eps_sb = const.tile([P, 1], F32, tag="eps")
        nc.vector.memset(eps_sb, EPS)
        bout_sb = const.tile([P, D], F32, tag="bout")
        nc.scalar.dma_start(out=bout_sb, in_=bo_d[0:1, :].to_broadcast((P, D)))

        aoT_bf = [ao_pool.tile([P, N], BF16, tag=f"ao{t}", name=f"ao{t}")
                  for t in range(NT_G)]

        # phase pools, strict LIFO: opened in reverse order of closing
        qk_cm = tc.tile_pool(name="qk", bufs=1)          # closes after attention
        qk_pool = qk_cm.__enter__()
        v_cm = tc.tile_pool(name="vext", bufs=1)         # closes after attention
        v_pool = v_cm.__enter__()
        xnT_cm = tc.tile_pool(name="xnT", bufs=1)        # closes after projections
        xnT_pool = xnT_cm.__enter__()
        psTR_cm = tc.tile_pool(name="psTR", bufs=2, space="PSUM")
        psTR = psTR_cm.__enter__()
        psQK_cm = tc.tile_pool(name="psQK", bufs=3, space="PSUM")
        psQK = psQK_cm.__enter__()
        psV_cm = tc.tile_pool(name="psV", bufs=3, space="PSUM")
        psV = psV_cm.__enter__()
        xn_cm = tc.tile_pool(name="xn", bufs=1)          # closes after transposes
        xn_pool = xn_cm.__enter__()
        wstage_cm = tc.tile_pool(name="wstage", bufs=2)  # closes after LN
        wstage = wstage_cm.__enter__()
        xstage_cm = tc.tile_pool(name="xstage", bufs=2)  # closes after LN
        xstage = xstage_cm.__enter__()

        # ---- weights: DMA f32, cast to bf16 ------------------------------
        def load_cast(dram, rows, cols, tagp):
            tiles = []
            for t in range(rows // P):
                st = wstage.tile([P, cols], F32, tag="wst")
                nc.scalar.dma_start(out=st, in_=dram[t * P:(t + 1) * P, :])
                bf = wpool.tile([P, cols], BF16, tag=f"{tagp}{t}")
                nc.gpsimd.tensor_copy(out=bf, in_=st)
                tiles.append(bf)
            return tiles

        wv_bf = load_cast(wv_d, D, GC, "wv")
        wq_bf = load_cast(wq_d, D, GC, "wq")
        wk_bf = load_cast(wk_d, D, GC, "wk")
        wo_bf = load_cast(wo_d, GC, D, "wo")

        # ---- LayerNorm: natural [i, c] layout, bn_stats over free dim ----
        xn_bf = []
        for i in range(NT_I):
            xs = xstage.tile([P, D], F32, tag="xst")
            eng = nc.sync if i % 2 == 0 else nc.gpsimd
            eng.dma_start(out=xs, in_=x_d[i * P:(i + 1) * P, :])
            stats = small.tile([P, 2, 6], F32, tag="stats")
            for sg in range(2):
                nc.vector.bn_stats(out=stats[:, sg, :], in_=xs[:, sg * 512:(sg + 1) * 512])
            mv = small.tile([P, 2], F32, tag="mv")
            nc.vector.bn_aggr(out=mv, in_=stats)
            std = small.tile([P, 1], F32, tag="std")
            nc.scalar.activation(out=std, in_=mv[:, 1:2], func=AF.Sqrt, bias=eps_sb)
            rstd = small.tile([P, 1], F32, tag="rstd")
            nc.vector.reciprocal(out=rstd, in_=std)
            nbias = small.tile([P, 1], F32, tag="nbias")
            nc.vector.scalar_tensor_tensor(nbias, mv[:, 0:1], -1.0, rstd,
                                           op0=mybir.AluOpType.mult,
                                           op1=mybir.AluOpType.mult)
            xt = xn_pool.tile([P, D], BF16, tag=f"xn{i}")
            nc.scalar.activation(out=xt, in_=xs, func=AF.Identity,
                                 bias=nbias, scale=rstd)
            xn_bf.append(xt)
        xstage_cm.__exit__(None, None, None)
        wstage_cm.__exit__(None, None, None)

        # ---- transpose xn -> xnT [c, i] via PE (i-major, fused with v) ---
        # xnT_all packs the 8 c-tiles side by side: segment ct covers
        # columns [ct*N, (ct+1)*N).  Transposing i-major lets the v
        # projection (which only needs column block i) start during LN.
        xnT_all = xnT_pool.tile([P, NT_C * N], BF16, tag="xnT", name="xnT")
        VW = HPC * (DH + 1)  # 520
        v_ext = []
        for i in range(NT_I):
            ps = psTR.tile([P, D], BF16, tag="ps_tr")
            for ct in range(NT_C):
                nc.tensor.transpose(ps[:, ct * P:(ct + 1) * P],
                                    xn_bf[i][:, ct * P:(ct + 1) * P],
                                    identity)
            nc.vector.tensor_copy(
                out=xnT_all[:, :].rearrange("p (ct i) -> p ct i", ct=NT_C)[:, :, i * P:(i + 1) * P],
                in_=ps[:].rearrange("p (ct i) -> p ct i", i=P))
            # v projection for this sequence block, with ones columns
            vt = v_pool.tile([P, VW], BF16, tag=f"v{i}", name=f"v{i}")
            nc.gpsimd.memset(vt, 1.0)
            psv = psV.tile([P, 512], F32, tag="ps_v")
            for ct in range(NT_C):
                nc.tensor.matmul(psv,
                                 xnT_all[:, ct * N + i * P:ct * N + (i + 1) * P],
                                 wv_bf[ct],
                                 start=(ct == 0), stop=(ct == NT_C - 1))
            nc.vector.tensor_copy(
                out=vt[:, 0:VW].rearrange("p (h e) -> p h e", h=HPC)[:, :, 0:DH],
                in_=psv[:].rearrange("p (h e) -> p h e", e=DH))
            v_ext.append(vt)
        xn_cm.__exit__(None, None, None)

        # ---- q/k projections --------------------------------------------
        def project_T(w_bf, tagp, m):
            # out[d_cols 128, i 2048] = (xn @ W)^T slice for m-tile, bf16
            ot = qk_pool.tile([P, N], BF16, tag=f"{tagp}{m}", name=f"{tagp}{m}")
            for nck in range(4):
                ps = psQK.tile([P, 512], F32, tag="ps_qk")
                for ct in range(NT_C):
                    nc.tensor.matmul(ps,
                                     w_bf[ct][:, m * P:(m + 1) * P],
                                     xnT_all[:, ct * N + nck * 512:ct * N + (nck + 1) * 512],
                                     start=(ct == 0), stop=(ct == NT_C - 1))
                nc.vector.tensor_copy(out=ot[:, nck * 512:(nck + 1) * 512], in_=ps)
            return ot

        qT_bf = [project_T(wq_bf, "qT", m) for m in range(NT_G)]
        kT_bf = [project_T(wk_bf, "kT", m) for m in range(NT_G)]
        psV_cm.__exit__(None, None, None)
        psQK_cm.__exit__(None, None, None)
        psTR_cm.__exit__(None, None, None)
        xnT_cm.__exit__(None, None, None)

        # ---- attention (i processed in two halves) -----------------------
        pt_cm = tc.tile_pool(name="pt", bufs=2)
        pt_pool = pt_cm.__enter__()
        rc_cm = tc.tile_pool(name="rc", bufs=2)
        rc_pool = rc_cm.__enter__()
        psS_cm = tc.tile_pool(name="psS", bufs=3, space="PSUM")
        psS = psS_cm.__enter__()
        psO_cm = tc.tile_pool(name="psO", bufs=1, space="PSUM")
        psO = psO_cm.__enter__()

        def emit_B(ph, pj):
            hh, ss = ph
            nc.tensor.matmul(ps_o_cur[ph][:, 0:512],
                             v_ext[pj][:, hh * (DH + 1):(hh + 1) * (DH + 1)],
                             pts_prev[pj][:, 0:512],
                             start=(pj == 0), stop=(pj == NT_I - 1),
                             skip_group_check=True)
            nc.tensor.matmul(ps_o_cur[ph][:, 512:1024],
                             v_ext[pj][:, hh * (DH + 1):(hh + 1) * (DH + 1)],
                             pts_prev[pj][:, 512:1024],
                             start=(pj == 0), stop=(pj == NT_I - 1),
                             skip_group_check=True)

        def emit_div(ph):
            hh, ss = ph
            ppo = (hh % 2) * DH
            pi0 = ss * IH
            po_t = ps_o_cur.pop(ph)
            nc.vector.tensor_copy(out=aoT_bf[hh // 2][ppo:ppo + DH, pi0:pi0 + IH],
                                  in_=po_t[0:DH, :])
            r = hh * 2 + ss
            zrow = rc_pool.tile([1, IH], F32, tag="zrow")
            nc.vector.tensor_copy(out=zrow, in_=po_t[DH:DH + 1, :])
            nc.sync.dma_start(out=zs_d[r:r + 1, :], in_=zrow)
            rb = rc_pool.tile([P, IH], F32, tag="rb")
            nc.sync.dma_start(out=rb[ppo:ppo + DH, :],
                              in_=zs_d[r:r + 1, :].to_broadcast((DH, IH)))
            nc.vector.reciprocal(out=rb[ppo:ppo + DH, :], in_=rb[ppo:ppo + DH, :])
            sl = aoT_bf[hh // 2][ppo:ppo + DH, pi0:pi0 + IH]
            nc.vector.tensor_mul(sl, sl, rb[ppo:ppo + DH, :])

        # software pipeline: while phase p's scores stream through PE+ACT,
        # phase p-1's attn@v accumulation interleaves on PE
        phases = [(h, s) for h in range(HPC) for s in range(2)]
        ps_o_cur = {}
        pts_prev, prev = None, None
        for ph in phases:
            h, s = ph
            qt = qT_bf[h // 2]
            kt = kT_bf[h // 2]
            po = (h % 2) * DH
            i0 = s * IH
            if prev is not None:
                ps_o_cur[prev] = psO.tile([DH + 1, IH], F32, tag="ps_o",
                                          name=f"ps_o{h}_{s}")
            pts = []
            for j in range(NT_I):
                pt = pt_pool.tile([P, IH], BF16, tag=f"pt{j}", name=f"pt{j}")
                ps = psS.tile([P, IH], F32, tag="ps_s")
                for nck in range(2):
                    nc.tensor.matmul(ps[:, nck * 512:(nck + 1) * 512],
                                     kt[po:po + DH, j * P:(j + 1) * P],
                                     qt[po:po + DH, i0 + nck * 512:i0 + (nck + 1) * 512],
                                     start=True, stop=True)
                nc.scalar.activation(out=pt, in_=ps, func=AF.Exp, scale=SCALE)
                if prev is not None:
                    emit_B(prev, j)
                pts.append(pt)
            if prev is not None:
                emit_div(prev)
            pts_prev, prev = pts, ph
        ps_o_cur[prev] = psO.tile([DH + 1, IH], F32, tag="ps_o", name="ps_o_last")
        for j in range(NT_I):
            emit_B(prev, j)
        emit_div(prev)

        psO_cm.__exit__(None, None, None)
        psS_cm.__exit__(None, None, None)
        rc_cm.__exit__(None, None, None)
        pt_cm.__exit__(None, None, None)
        v_cm.__exit__(None, None, None)
        qk_cm.__exit__(None, None, None)

        # ---- partial out-projection + bias ------------------------------
        y_cm = tc.tile_pool(name="ybuf", bufs=3)
        y_pool = y_cm.__enter__()
        psY_cm = tc.tile_pool(name="psY", bufs=4, space="PSUM")
        psY = psY_cm.__enter__()
        for i in range(NT_I):
            ys = y_pool.tile([P, D], F32, tag="ys")
            for nck in range(2):
                ps = psY.tile([P, 512], F32, tag="ps_y")
                for t in range(NT_G):
                    nc.tensor.matmul(ps,
                                     aoT_bf[t][:, i * P:(i + 1) * P],
                                     wo_bf[t][:, nck * 512:(nck + 1) * 512],
                                     start=(t == 0), stop=(t == NT_G - 1))
                nc.vector.tensor_add(ys[:, nck * 512:(nck + 1) * 512], ps,
                                     bout_sb[:, nck * 512:(nck + 1) * 512])
            nc.sync.dma_start(out=out_d[i * P:(i + 1) * P, :], in_=ys)
        psY_cm.__exit__(None, None, None)
        y_cm.__exit__(None, None, None)

    nc.compile()
    return nc


def kernel(x, gamma, Wq, Wkv, Wout, bout, _trace=False, _tmpdir=None):
    global _CACHED_NC, LAST_EXEC_NS, LAST_TRACE
    x = np.asarray(x, dtype=np.float32)
    gamma = np.asarray(gamma, dtype=np.float32)
    Wq = np.asarray(Wq, dtype=np.float32)
    Wkv = np.asarray(Wkv, dtype=np.float32)
    Wout = np.asarray(Wout, dtype=np.float32)
    bout = np.asarray(bout, dtype=np.float32)

    # fold LN gamma into the projection weights (exact)
    Wqg = gamma[:, None] * Wq
    Wk = gamma[:, None] * Wkv[:, :D]
    Wv = gamma[:, None] * Wkv[:, D:]
    zeros_b = np.zeros((1, D), dtype=np.float32)

    in_maps = []
    for c in range(8):
        b, g = divmod(c, 2)
        sl = slice(g * GC, (g + 1) * GC)
        in_maps.append({
            "x": np.ascontiguousarray(x[b]),
            "wq": np.ascontiguousarray(Wqg[:, sl]),
            "wk": np.ascontiguousarray(Wk[:, sl]),
            "wv": np.ascontiguousarray(Wv[:, sl]),
            "wout": np.ascontiguousarray(Wout[sl, :]),
            "bout": bout.reshape(1, D) if g == 0 else zeros_b,
        })

    if _CACHED_NC is None:
        _CACHED_NC = build_nc()
    nc = _CACHED_NC

    kw = {}
    if _trace:
        import concourse.bass_utils as bu
        bu.upload_artifacts = lambda tmpdir: "not-uploaded"
        kw = dict(trace=True, tmpdir=_tmpdir)
    res = run_bass_kernel_spmd(nc, in_maps, core_ids=list(range(8)), **kw)
    LAST_EXEC_NS = res.exec_time_ns
    LAST_TRACE = getattr(res, "instructions_and_trace", None)

    out = np.empty((B, N, D), dtype=np.float32)
    for b in range(B):
        out[b] = res.results[2 * b]["out"] + res.results[2 * b + 1]["out"]
    return out


# revision 22
# speedup vs baseline: 1.1681x; 1.1681x over previous
"""Distributed Trainium2 kernel for a pre-LN single attention block.

Reference computation (dims hardcoded):
    x: [4, 2048, 1024]; LN(x) -> q = xn@Wq, kv = xn@Wkv; 16 heads x 64;
    softmax(q k^T / 8) v ; out proj [1024,1024] + bias.

Sharding over 8 NeuronCores: core c handles batch b = c//2 and head
group g = c%2 (8 heads each).  Each core computes LN(x[b]) (duplicated
across the pair, cheap), its 512-wide q/k/v projection slices, its 8
attention heads and a PARTIAL out-projection (contraction over its 512
inner columns).  The two partials per batch are summed on the host
during unshard -- no on-chip collectives.  gamma is folded into the
projection weights on the host; bout is fed only to g==0 cores (zeros
to g==1) so the SPMD graph is identical on all cores.

All matmuls run in bf16 with f32 PSUM accumulation.  Attention scores
are built transposed (scoresT[j, i]) so softmax's exp lands in the
layout attn@v needs; the row-sum for softmax comes from an appended
ones-column in v; the max-subtraction is skipped (scores ~ N(0,1) after
LN, |s| < ~5, exp is safe).  The query/sequence axis of attention is
processed in two halves to halve the probability-tile SBUF footprint.
"""

import numpy as np
from contextlib import ExitStack

import concourse.bass as bass
import concourse.bacc as bacc_mod
import concourse.mybir as mybir
import concourse.tile as tile
from concourse.bass_utils import run_bass_kernel_spmd
from concourse.masks import make_identity

F32 = mybir.dt.float32
BF16 = mybir.dt.bfloat16
AF = mybir.ActivationFunctionType

B = 4
N = 2048          # sequence length
D = 1024          # model dim
GC = 512          # per-core inner columns (8 heads x 64)
DH = 64           # head dim
HPC = 8           # heads per core
P = 128
NT_I = N // P     # 16 sequence tiles
NT_C = D // P     # 8 model-dim tiles
NT_G = GC // P    # 4 inner tiles
IH = N // 2       # attention i-half length (1024)
SCALE = DH ** -0.5
EPS = 1e-5

LAST_EXEC_NS = None
LAST_TRACE = None
_CACHED_NC = None


def build_nc():
    nc = bacc_mod.Bacc()
    x_d = nc.declare_dram_parameter("x", [N, D], F32, isOutput=False)
    wq_d = nc.declare_dram_parameter("wq", [D, GC], F32, isOutput=False)
    wk_d = nc.declare_dram_parameter("wk", [D, GC], F32, isOutput=False)
    wv_d = nc.declare_dram_parameter("wv", [D, GC], F32, isOutput=False)
    wo_d = nc.declare_dram_parameter("wout", [GC, D], F32, isOutput=False)
    bo_d = nc.declare_dram_parameter("bout", [1, D], F32, isOutput=False)
    out_d = nc.declare_dram_parameter("out", [N, D], F32, isOutput=True)
    zs_d = nc.dram_tensor("zscratch", [HPC * 2, IH], F32)

    ctx = ExitStack()
    with ctx:
        tc = ctx.enter_context(tile.TileContext(nc))

        # outer pools, live for the whole kernel
        const = ctx.enter_context(tc.tile_pool(name="const", bufs=1))
        wpool = ctx.enter_context(tc.tile_pool(name="wpool", bufs=1))
        small = ctx.enter_context(tc.tile_pool(name="small", bufs=4))
        ao_pool = ctx.enter_context(tc.tile_pool(name="aoT", bufs=1))

        identity = const.tile([P, P], BF16, tag="identity")
        make_identity(nc, identity)
        eps_sb = const.tile([P, 1], F32, tag="eps")
        nc.vector.memset(eps_sb, EPS)
        bout_sb = const.tile([P, D], F32, tag="bout")
        nc.scalar.dma_start(out=bout_sb, in_=bo_d[0:1, :].to_broadcast((P, D)))

        aoT_bf = [ao_pool.tile([P, N], BF16, tag=f"ao{t}", name=f"ao{t}")
                  for t in range(NT_G)]

        # phase pools, strict LIFO: opened in reverse order of closing
        qk_cm = tc.tile_pool(name="qk", bufs=1)          # closes after attention
        qk_pool = qk_cm.__enter__()
        v_cm = tc.tile_pool(name="vext", bufs=1)         # closes after attention
        v_pool = v_cm.__enter__()
        xnT_cm = tc.tile_pool(name="xnT", bufs=1)        # closes after projections
        xnT_pool = xnT_cm.__enter__()
        psTR_cm = tc.tile_pool(name="psTR", bufs=2, space="PSUM")
        psTR = psTR_cm.__enter__()
        psQK_cm = tc.tile_pool(name="psQK", bufs=3, space="PSUM")
        psQK = psQK_cm.__enter__()
        psV_cm = tc.tile_pool(name="psV", bufs=3, space="PSUM")
        psV = psV_cm.__enter__()
        xn_cm = tc.tile_pool(name="xn", bufs=1)          # closes after transposes
        xn_pool = xn_cm.__enter__()
        wstage_cm = tc.tile_pool(name="wstage", bufs=2)  # closes after LN
        wstage = wstage_cm.__enter__()
        xstage_cm = tc.tile_pool(name="xstage", bufs=2)  # closes after LN
        xstage = xstage_cm.__enter__()

        # ---- weights: DMA f32, cast to bf16 ------------------------------
        def load_cast(dram, rows, cols, tagp):
            tiles = []
            for t in range(rows // P):
                st = wstage.tile([P, cols], F32, tag="wst")
                nc.scalar.dma_start(out=st, in_=dram[t * P:(t + 1) * P, :])
                bf = wpool.tile([P, cols], BF16, tag=f"{tagp}{t}")
                nc.gpsimd.tensor_copy(out=bf, in_=st)
                tiles.append(bf)
            return tiles

        wv_bf = load_cast(wv_d, D, GC, "wv")
        wq_bf = load_cast(wq_d, D, GC, "wq")
        wk_bf = load_cast(wk_d, D, GC, "wk")
        wo_bf = load_cast(wo_d, GC, D, "wo")

        # ---- LayerNorm: natural [i, c] layout, bn_stats over free dim ----
        xn_bf = []
        for i in range(NT_I):
            xs = xstage.tile([P, D], F32, tag="xst")
            eng = nc.sync if i % 2 == 0 else nc.gpsimd
            eng.dma_start(out=xs, in_=x_d[i * P:(i + 1) * P, :])
            stats = small.tile([P, 2, 6], F32, tag="stats")
            for sg in range(2):
                nc.vector.bn_stats(out=stats[:, sg, :], in_=xs[:, sg * 512:(sg + 1) * 512])
            mv = small.tile([P, 2], F32, tag="mv")
            nc.vector.bn_aggr(out=mv, in_=stats)
            std = small.tile([P, 1], F32, tag="std")
            nc.scalar.activation(out=std, in_=mv[:, 1:2], func=AF.Sqrt, bias=eps_sb)
            rstd = small.tile([P, 1], F32, tag="rstd")
            nc.vector.reciprocal(out=rstd, in_=std)
            nbias = small.tile([P, 1], F32, tag="nbias")
            nc.vector.scalar_tensor_tensor(nbias, mv[:, 0:1], -1.0, rstd,
                                           op0=mybir.AluOpType.mult,
                                           op1=mybir.AluOpType.mult)
            xt = xn_pool.tile([P, D], BF16, tag=f"xn{i}")
            nc.scalar.activation(out=xt, in_=xs, func=AF.Identity,
                                 bias=nbias, scale=rstd)
            xn_bf.append(xt)
        xstage_cm.__exit__(None, None, None)
        wstage_cm.__exit__(None, None, None)

        # ---- transpose xn -> xnT [c, i] via PE (i-major, fused with v) ---
        # xnT_all packs the 8 c-tiles side by side: segment ct covers
        # columns [ct*N, (ct+1)*N).  Transposing i-major lets the v
        # projection (which only needs column block i) start during LN.
        xnT_all = xnT_pool.tile([P, NT_C * N], BF16, tag="xnT", name="xnT")
        VW = HPC * (DH + 1)  # 520
        v_ext = []
        for i in range(NT_I):
            ps = psTR.tile([P, D], BF16, tag="ps_tr")
            for ct in range(NT_C):
                nc.tensor.transpose(ps[:, ct * P:(ct + 1) * P],
                                    xn_bf[i][:, ct * P:(ct + 1) * P],
                                    identity)
            nc.vector.tensor_copy(
                out=xnT_all[:, :].rearrange("p (ct i) -> p ct i", ct=NT_C)[:, :, i * P:(i + 1) * P],
                in_=ps[:].rearrange("p (ct i) -> p ct i", i=P))
            # v projection for this sequence block, with ones columns
            vt = v_pool.tile([P, VW], BF16, tag=f"v{i}", name=f"v{i}")
            nc.gpsimd.memset(vt, 1.0)
            psv = psV.tile([P, 512], F32, tag="ps_v")
            for ct in range(NT_C):
                nc.tensor.matmul(psv,
                                 xnT_all[:, ct * N + i * P:ct * N + (i + 1) * P],
                                 wv_bf[ct],
                                 start=(ct == 0), stop=(ct == NT_C - 1))
            nc.vector.tensor_copy(
                out=vt[:, 0:VW].rearrange("p (h e) -> p h e", h=HPC)[:, :, 0:DH],
                in_=psv[:].rearrange("p (h e) -> p h e", e=DH))
            v_ext.append(vt)
        xn_cm.__exit__(None, None, None)

        # ---- q/k projections --------------------------------------------
        def project_T(w_bf, tagp, m):
            # out[d_cols 128, i 2048] = (xn @ W)^T slice for m-tile, bf16
            ot = qk_pool.tile([P, N], BF16, tag=f"{tagp}{m}", name=f"{tagp}{m}")
            for nck in range(4):
                ps = psQK.tile([P, 512], F32, tag="ps_qk")
                for ct in range(NT_C):
                    nc.tensor.matmul(ps,
                                     w_bf[ct][:, m * P:(m + 1) * P],
                                     xnT_all[:, ct * N + nck * 512:ct * N + (nck + 1) * 512],
                                     start=(ct == 0), stop=(ct == NT_C - 1))
                nc.vector.tensor_copy(out=ot[:, nck * 512:(nck + 1) * 512], in_=ps)
            return ot

        qT_bf = [project_T(wq_bf, "qT", m) for m in range(NT_G)]
        kT_bf = [project_T(wk_bf, "kT", m) for m in range(NT_G)]
        psV_cm.__exit__(None, None, None)
        psQK_cm.__exit__(None, None, None)
        psTR_cm.__exit__(None, None, None)
        xnT_cm.__exit__(None, None, None)

        # ---- attention (i processed in two halves) -----------------------
        pt_cm = tc.tile_pool(name="pt", bufs=2)
        pt_pool = pt_cm.__enter__()
        rc_cm = tc.tile_pool(name="rc", bufs=2)
        rc_pool = rc_cm.__enter__()
        psS_cm = tc.tile_pool(name="psS", bufs=3, space="PSUM")
        psS = psS_cm.__enter__()
        psO_cm = tc.tile_pool(name="psO", bufs=1, space="PSUM")
        psO = psO_cm.__enter__()

        for h in range(HPC):
            qt = qT_bf[h // 2]
            kt = kT_bf[h // 2]
            po = (h % 2) * DH  # partition offset of this head in its tile
            for half in range(2):
                i0 = half * IH
                po_t = psO.tile([DH + 1, IH], F32, tag="ps_o")
                for j in range(NT_I):
                    pt = pt_pool.tile([P, IH], BF16, tag=f"pt{j}", name=f"pt{j}")
                    ps = psS.tile([P, IH], F32, tag="ps_s")
                    for nck in range(2):
                        nc.tensor.matmul(ps[:, nck * 512:(nck + 1) * 512],
                                         kt[po:po + DH, j * P:(j + 1) * P],
                                         qt[po:po + DH, i0 + nck * 512:i0 + (nck + 1) * 512],
                                         start=True, stop=True)
                    nc.scalar.activation(out=pt, in_=ps, func=AF.Exp, scale=SCALE)
                    for nck in range(2):
                        nc.tensor.matmul(po_t[:, nck * 512:(nck + 1) * 512],
                                         v_ext[j][:, h * (DH + 1):(h + 1) * (DH + 1)],
                                         pt[:, nck * 512:(nck + 1) * 512],
                                         start=(j == 0), stop=(j == NT_I - 1),
                                         skip_group_check=True)
                # evict unnormalized head output + denominator row, then
                # normalize via a DRAM-broadcast reciprocal-multiply
                nc.vector.tensor_copy(out=aoT_bf[h // 2][po:po + DH, i0:i0 + IH],
                                      in_=po_t[0:DH, :])
                r = h * 2 + half
                zrow = rc_pool.tile([1, IH], F32, tag="zrow")
                nc.vector.tensor_copy(out=zrow, in_=po_t[DH:DH + 1, :])
                nc.sync.dma_start(out=zs_d[r:r + 1, :], in_=zrow)
                rb = rc_pool.tile([P, IH], F32, tag="rb")
                nc.sync.dma_start(out=rb[po:po + DH, :],
                                  in_=zs_d[r:r + 1, :].to_broadcast((DH, IH)))
                nc.vector.reciprocal(out=rb[po:po + DH, :], in_=rb[po:po + DH, :])
                sl = aoT_bf[h // 2][po:po + DH, i0:i0 + IH]
                nc.vector.tensor_mul(sl, sl, rb[po:po + DH, :])

        psO_cm.__exit__(None, None, None)
        psS_cm.__exit__(None, None, None)
        rc_cm.__exit__(None, None, None)
        pt_cm.__exit__(None, None, None)
        v_cm.__exit__(None, None, None)
        qk_cm.__exit__(None, None, None)

        # ---- partial out-projection + bias ------------------------------
        y_cm = tc.tile_pool(name="ybuf", bufs=3)
        y_pool = y_cm.__enter__()
        psY_cm = tc.tile_pool(name="psY", bufs=4, space="PSUM")
        psY = psY_cm.__enter__()
        for i in range(NT_I):
            ys = y_pool.tile([P, D], F32, tag="ys")
            for nck in range(2):
                ps = psY.tile([P, 512], F32, tag="ps_y")
                for t in range(NT_G):
                    nc.tensor.matmul(ps,
                                     aoT_bf[t][:, i * P:(i + 1) * P],
                                     wo_bf[t][:, nck * 512:(nck + 1) * 512],
                                     start=(t == 0), stop=(t == NT_G - 1))
                nc.vector.tensor_add(ys[:, nck * 512:(nck + 1) * 512], ps,
                                     bout_sb[:, nck * 512:(nck + 1) * 512])
            nc.sync.dma_start(out=out_d[i * P:(i + 1) * P, :], in_=ys)
        psY_cm.__exit__(None, None, None)
        y_cm.__exit__(None, None, None)

    nc.compile()
    return nc


def kernel(x, gamma, Wq, Wkv, Wout, bout, _trace=False, _tmpdir=None):
    global _CACHED_NC, LAST_EXEC_NS, LAST_TRACE
    x = np.asarray(x, dtype=np.float32)
    gamma = np.asarray(gamma, dtype=np.float32)
    Wq = np.asarray(Wq, dtype=np.float32)
    Wkv = np.asarray(Wkv, dtype=np.float32)
    Wout = np.asarray(Wout, dtype=np.float32)
    bout = np.asarray(bout, dtype=np.float32)

    # fold LN gamma into the projection weights (exact)
    Wqg = gamma[:, None] * Wq
    Wk = gamma[:, None] * Wkv[:, :D]
    Wv = gamma[:, None] * Wkv[:, D:]
    zeros_b = np.zeros((1, D), dtype=np.float32)

    in_maps = []
    for c in range(8):
        b, g = divmod(c, 2)
        sl = slice(g * GC, (g + 1) * GC)
        in_maps.append({
            "x": np.ascontiguousarray(x[b]),
            "wq": np.ascontiguousarray(Wqg[:, sl]),
            "wk": np.ascontiguousarray(Wk[:, sl]),
            "wv": np.ascontiguousarray(Wv[:, sl]),
            "wout": np.ascontiguousarray(Wout[sl, :]),
            "bout": bout.reshape(1, D) if g == 0 else zeros_b,
        })

    if _CACHED_NC is None:
        _CACHED_NC = build_nc()
    nc = _CACHED_NC

    kw = {}
    if _trace:
        import concourse.bass_utils as bu
        bu.upload_artifacts = lambda tmpdir: "not-uploaded"
        kw = dict(trace=True, tmpdir=_tmpdir)
    res = run_bass_kernel_spmd(nc, in_maps, core_ids=list(range(8)), **kw)
    LAST_EXEC_NS = res.exec_time_ns
    LAST_TRACE = getattr(res, "instructions_and_trace", None)

    out = np.empty((B, N, D), dtype=np.float32)
    for b in range(B):
        out[b] = res.results[2 * b]["out"] + res.results[2 * b + 1]["out"]
    return out


# revision 23
# speedup vs baseline: 1.2068x; 1.0331x over previous
"""Distributed Trainium2 kernel for a pre-LN single attention block.

Reference computation (dims hardcoded):
    x: [4, 2048, 1024]; LN(x) -> q = xn@Wq, kv = xn@Wkv; 16 heads x 64;
    softmax(q k^T / 8) v ; out proj [1024,1024] + bias.

Sharding over 8 NeuronCores: core c handles batch b = c//2 and head
group g = c%2 (8 heads each).  Each core computes LN(x[b]) (duplicated
across the pair, cheap), its 512-wide q/k/v projection slices, its 8
attention heads and a PARTIAL out-projection (contraction over its 512
inner columns).  The two partials per batch are summed on the host
during unshard -- no on-chip collectives.  gamma is folded into the
projection weights on the host; bout is fed only to g==0 cores (zeros
to g==1) so the SPMD graph is identical on all cores.

All matmuls run in bf16 with f32 PSUM accumulation.  Attention scores
are built transposed (scoresT[j, i]) so softmax's exp lands in the
layout attn@v needs; the row-sum for softmax comes from an appended
ones-column in v; the max-subtraction is skipped (scores ~ N(0,1) after
LN, |s| < ~5, exp is safe).  The query/sequence axis of attention is
processed in two halves to halve the probability-tile SBUF footprint.
"""

import numpy as np
from contextlib import ExitStack

import concourse.bass as bass
import concourse.bacc as bacc_mod
import concourse.mybir as mybir
import concourse.tile as tile
from concourse.bass_utils import run_bass_kernel_spmd
from concourse.masks import make_identity

F32 = mybir.dt.float32
BF16 = mybir.dt.bfloat16
AF = mybir.ActivationFunctionType

B = 4
N = 2048          # sequence length
D = 1024          # model dim
GC = 512          # per-core inner columns (8 heads x 64)
DH = 64           # head dim
HPC = 8           # heads per core
P = 128
NT_I = N // P     # 16 sequence tiles
NT_C = D // P     # 8 model-dim tiles
NT_G = GC // P    # 4 inner tiles
IH = N // 2       # attention i-half length (1024)
SCALE = DH ** -0.5
EPS = 1e-5

LAST_EXEC_NS = None
LAST_TRACE = None
_CACHED_NC = None


def build_nc():
    nc = bacc_mod.Bacc()
    x_d = nc.declare_dram_parameter("x", [N, D], F32, isOutput=False)
    wq_d = nc.declare_dram_parameter("wq", [D, GC], F32, isOutput=False)
    wk_d = nc.declare_dram_parameter("wk", [D, GC], F32, isOutput=False)
    wv_d = nc.declare_dram_parameter("wv", [D, GC], F32, isOutput=False)
    wo_d = nc.declare_dram_parameter("wout", [GC, D], F32, isOutput=False)
    bo_d = nc.declare_dram_parameter("bout", [1, D], F32, isOutput=False)
    out_d = nc.declare_dram_parameter("out", [N, D], F32, isOutput=True)
    zs_d = nc.dram_tensor("zscratch", [HPC * 2, IH], F32)

    ctx = ExitStack()
    with ctx:
        tc = ctx.enter_context(tile.TileContext(nc))

        # outer pools, live for the whole kernel
        const = ctx.enter_context(tc.tile_pool(name="const", bufs=1))
        wpool = ctx.enter_context(tc.tile_pool(name="wpool", bufs=1))
        small = ctx.enter_context(tc.tile_pool(name="small", bufs=4))
        ao_pool = ctx.enter_context(tc.tile_pool(name="aoT", bufs=1))

        identity = const.tile([P, P], BF16, tag="identity")
        make_identity(nc, identity)
        eps_sb = const.tile([P, 1], F32, tag="eps")
        nc.vector.memset(eps_sb, EPS)
        bout_sb = const.tile([P, D], F32, tag="bout")
        nc.scalar.dma_start(out=bout_sb, in_=bo_d[0:1, :].to_broadcast((P, D)))

        aoT_bf = [ao_pool.tile([P, N], BF16, tag=f"ao{t}", name=f"ao{t}")
                  for t in range(NT_G)]

        # phase pools, strict LIFO: opened in reverse order of closing
        qk_cm = tc.tile_pool(name="qk", bufs=1)          # closes after attention
        qk_pool = qk_cm.__enter__()
        v_cm = tc.tile_pool(name="vext", bufs=1)         # closes after attention
        v_pool = v_cm.__enter__()
        xnT_cm = tc.tile_pool(name="xnT", bufs=1)        # closes after projections
        xnT_pool = xnT_cm.__enter__()
        psTR_cm = tc.tile_pool(name="psTR", bufs=2, space="PSUM")
        psTR = psTR_cm.__enter__()
        psQK_cm = tc.tile_pool(name="psQK", bufs=3, space="PSUM")
        psQK = psQK_cm.__enter__()
        psV_cm = tc.tile_pool(name="psV", bufs=3, space="PSUM")
        psV = psV_cm.__enter__()
        xn_cm = tc.tile_pool(name="xn", bufs=1)          # closes after transposes
        xn_pool = xn_cm.__enter__()
        wstage_cm = tc.tile_pool(name="wstage", bufs=2)  # closes after LN
        wstage = wstage_cm.__enter__()
        xstage_cm = tc.tile_pool(name="xstage", bufs=2)  # closes after LN
        xstage = xstage_cm.__enter__()

        # ---- weights: DMA f32, cast to bf16 ------------------------------
        def load_cast(dram, rows, cols, tagp):
            tiles = []
            for t in range(rows // P):
                st = wstage.tile([P, cols], F32, tag="wst")
                nc.scalar.dma_start(out=st, in_=dram[t * P:(t + 1) * P, :])
                bf = wpool.tile([P, cols], BF16, tag=f"{tagp}{t}")
                nc.vector.tensor_copy(out=bf, in_=st)
                tiles.append(bf)
            return tiles

        wv_bf = load_cast(wv_d, D, GC, "wv")
        wq_bf = load_cast(wq_d, D, GC, "wq")
        wk_bf = load_cast(wk_d, D, GC, "wk")
        wo_bf = load_cast(wo_d, GC, D, "wo")

        # ---- LayerNorm: natural [i, c] layout, bn_stats over free dim ----
        xn_bf = []
        for i in range(NT_I):
            xs = xstage.tile([P, D], F32, tag="xst")
            eng = nc.sync if i % 2 == 0 else nc.gpsimd
            eng.dma_start(out=xs, in_=x_d[i * P:(i + 1) * P, :])
            stats = small.tile([P, 2, 6], F32, tag="stats")
            for sg in range(2):
                nc.vector.bn_stats(out=stats[:, sg, :], in_=xs[:, sg * 512:(sg + 1) * 512])
            mv = small.tile([P, 2], F32, tag="mv")
            nc.vector.bn_aggr(out=mv, in_=stats)
            std = small.tile([P, 1], F32, tag="std")
            nc.scalar.activation(out=std, in_=mv[:, 1:2], func=AF.Sqrt, bias=eps_sb)
            rstd = small.tile([P, 1], F32, tag="rstd")
            nc.vector.reciprocal(out=rstd, in_=std)
            nbias = small.tile([P, 1], F32, tag="nbias")
            nc.vector.scalar_tensor_tensor(nbias, mv[:, 0:1], -1.0, rstd,
                                           op0=mybir.AluOpType.mult,
                                           op1=mybir.AluOpType.mult)
            xt = xn_pool.tile([P, D], BF16, tag=f"xn{i}")
            nc.scalar.activation(out=xt, in_=xs, func=AF.Identity,
                                 bias=nbias, scale=rstd)
            xn_bf.append(xt)
        xstage_cm.__exit__(None, None, None)
        wstage_cm.__exit__(None, None, None)

        # ---- transpose xn -> xnT [c, i] via PE (i-major, fused with v) ---
        # xnT_all packs the 8 c-tiles side by side: segment ct covers
        # columns [ct*N, (ct+1)*N).  Transposing i-major lets the v
        # projection (which only needs column block i) start during LN.
        xnT_all = xnT_pool.tile([P, NT_C * N], BF16, tag="xnT", name="xnT")
        VW = HPC * (DH + 1)  # 520
        v_ext = []
        for i in range(NT_I):
            ps = psTR.tile([P, D], BF16, tag="ps_tr")
            for ct in range(NT_C):
                nc.tensor.transpose(ps[:, ct * P:(ct + 1) * P],
                                    xn_bf[i][:, ct * P:(ct + 1) * P],
                                    identity)
            nc.vector.tensor_copy(
                out=xnT_all[:, :].rearrange("p (ct i) -> p ct i", ct=NT_C)[:, :, i * P:(i + 1) * P],
                in_=ps[:].rearrange("p (ct i) -> p ct i", i=P))
            # v projection for this sequence block, with ones columns
            vt = v_pool.tile([P, VW], BF16, tag=f"v{i}", name=f"v{i}")
            nc.vector.memset(vt, 1.0)
            psv = psV.tile([P, 512], F32, tag="ps_v")
            for ct in range(NT_C):
                nc.tensor.matmul(psv,
                                 xnT_all[:, ct * N + i * P:ct * N + (i + 1) * P],
                                 wv_bf[ct],
                                 start=(ct == 0), stop=(ct == NT_C - 1))
            nc.vector.tensor_copy(
                out=vt[:, 0:VW].rearrange("p (h e) -> p h e", h=HPC)[:, :, 0:DH],
                in_=psv[:].rearrange("p (h e) -> p h e", e=DH))
            v_ext.append(vt)
        xn_cm.__exit__(None, None, None)

        # ---- q/k projections --------------------------------------------
        def project_T(w_bf, tagp, m):
            # out[d_cols 128, i 2048] = (xn @ W)^T slice for m-tile, bf16
            ot = qk_pool.tile([P, N], BF16, tag=f"{tagp}{m}", name=f"{tagp}{m}")
            for nck in range(4):
                ps = psQK.tile([P, 512], F32, tag="ps_qk")
                for ct in range(NT_C):
                    nc.tensor.matmul(ps,
                                     w_bf[ct][:, m * P:(m + 1) * P],
                                     xnT_all[:, ct * N + nck * 512:ct * N + (nck + 1) * 512],
                                     start=(ct == 0), stop=(ct == NT_C - 1))
                nc.vector.tensor_copy(out=ot[:, nck * 512:(nck + 1) * 512], in_=ps)
            return ot

        qT_bf = [project_T(wq_bf, "qT", m) for m in range(NT_G)]
        kT_bf = [project_T(wk_bf, "kT", m) for m in range(NT_G)]
        psV_cm.__exit__(None, None, None)
        psQK_cm.__exit__(None, None, None)
        psTR_cm.__exit__(None, None, None)
        xnT_cm.__exit__(None, None, None)

        # ---- attention (i processed in two halves) -----------------------
        pt_cm = tc.tile_pool(name="pt", bufs=2)
        pt_pool = pt_cm.__enter__()
        rc_cm = tc.tile_pool(name="rc", bufs=2)
        rc_pool = rc_cm.__enter__()
        psS_cm = tc.tile_pool(name="psS", bufs=3, space="PSUM")
        psS = psS_cm.__enter__()
        psO_cm = tc.tile_pool(name="psO", bufs=1, space="PSUM")
        psO = psO_cm.__enter__()

        for h in range(HPC):
            qt = qT_bf[h // 2]
            kt = kT_bf[h // 2]
            po = (h % 2) * DH  # partition offset of this head in its tile
            for half in range(2):
                i0 = half * IH
                po_t = psO.tile([DH + 1, IH], F32, tag="ps_o")
                for j in range(NT_I):
                    pt = pt_pool.tile([P, IH], BF16, tag=f"pt{j}", name=f"pt{j}")
                    ps = psS.tile([P, IH], F32, tag="ps_s")
                    for nck in range(2):
                        nc.tensor.matmul(ps[:, nck * 512:(nck + 1) * 512],
                                         kt[po:po + DH, j * P:(j + 1) * P],
                                         qt[po:po + DH, i0 + nck * 512:i0 + (nck + 1) * 512],
                                         start=True, stop=True)
                    nc.scalar.activation(out=pt, in_=ps, func=AF.Exp, scale=SCALE)
                    for nck in range(2):
                        nc.tensor.matmul(po_t[:, nck * 512:(nck + 1) * 512],
                                         v_ext[j][:, h * (DH + 1):(h + 1) * (DH + 1)],
                                         pt[:, nck * 512:(nck + 1) * 512],
                                         start=(j == 0), stop=(j == NT_I - 1),
                                         skip_group_check=True)
                # evict unnormalized head output + denominator row, then
                # normalize via a DRAM-broadcast reciprocal-multiply
                nc.vector.tensor_copy(out=aoT_bf[h // 2][po:po + DH, i0:i0 + IH],
                                      in_=po_t[0:DH, :])
                r = h * 2 + half
                zrow = rc_pool.tile([1, IH], F32, tag="zrow")
                nc.vector.tensor_copy(out=zrow, in_=po_t[DH:DH + 1, :])
                nc.sync.dma_start(out=zs_d[r:r + 1, :], in_=zrow)
                rb = rc_pool.tile([P, IH], F32, tag="rb")
                nc.sync.dma_start(out=rb[po:po + DH, :],
                                  in_=zs_d[r:r + 1, :].to_broadcast((DH, IH)))
                nc.vector.reciprocal(out=rb[po:po + DH, :], in_=rb[po:po + DH, :])
                sl = aoT_bf[h // 2][po:po + DH, i0:i0 + IH]
                nc.vector.tensor_mul(sl, sl, rb[po:po + DH, :])

        psO_cm.__exit__(None, None, None)
        psS_cm.__exit__(None, None, None)
        rc_cm.__exit__(None, None, None)
        pt_cm.__exit__(None, None, None)
        v_cm.__exit__(None, None, None)
        qk_cm.__exit__(None, None, None)

        # ---- partial out-projection + bias ------------------------------
        y_cm = tc.tile_pool(name="ybuf", bufs=3)
        y_pool = y_cm.__enter__()
        psY_cm = tc.tile_pool(name="psY", bufs=4, space="PSUM")
        psY = psY_cm.__enter__()
        for i in range(NT_I):
            ys = y_pool.tile([P, D], F32, tag="ys")
            for nck in range(2):
                ps = psY.tile([P, 512], F32, tag="ps_y")
                for t in range(NT_G):
                    nc.tensor.matmul(ps,
                                     aoT_bf[t][:, i * P:(i + 1) * P],
                                     wo_bf[t][:, nck * 512:(nck + 1) * 512],
                                     start=(t == 0), stop=(t == NT_G - 1))
                nc.vector.tensor_add(ys[:, nck * 512:(nck + 1) * 512], ps,
                                     bout_sb[:, nck * 512:(nck + 1) * 512])
            nc.sync.dma_start(out=out_d[i * P:(i + 1) * P, :], in_=ys)
        psY_cm.__exit__(None, None, None)
        y_cm.__exit__(None, None, None)

    nc.compile()
    return nc


def kernel(x, gamma, Wq, Wkv, Wout, bout, _trace=False, _tmpdir=None):
    global _CACHED_NC, LAST_EXEC_NS, LAST_TRACE
    x = np.asarray(x, dtype=np.float32)
    gamma = np.asarray(gamma, dtype=np.float32)
    Wq = np.asarray(Wq, dtype=np.float32)
    Wkv = np.asarray(Wkv, dtype=np.float32)
    Wout = np.asarray(Wout, dtype=np.float32)
    bout = np.asarray(bout, dtype=np.float32)

    # fold LN gamma into the projection weights (exact)
    Wqg = gamma[:, None] * Wq
    Wk = gamma[:, None] * Wkv[:, :D]
    Wv = gamma[:, None] * Wkv[:, D:]
    zeros_b = np.zeros((1, D), dtype=np.float32)

    in_maps = []
    for c in range(8):
        b, g = divmod(c, 2)
        sl = slice(g * GC, (g + 1) * GC)
        in_maps.append({
            "x": np.ascontiguousarray(x[b]),
            "wq": np.ascontiguousarray(Wqg[:, sl]),
            "wk": np.ascontiguousarray(Wk[:, sl]),
            "wv": np.ascontiguousarray(Wv[:, sl]),
            "wout": np.ascontiguousarray(Wout[sl, :]),
            "bout": bout.reshape(1, D) if g == 0 else zeros_b,
        })

    if _CACHED_NC is None:
        _CACHED_NC = build_nc()
    nc = _CACHED_NC

    kw = {}
    if _trace:
        import concourse.bass_utils as bu
        bu.upload_artifacts = lambda tmpdir: "not-uploaded"
        kw = dict(trace=True, tmpdir=_tmpdir)
    res = run_bass_kernel_spmd(nc, in_maps, core_ids=list(range(8)), **kw)
    LAST_EXEC_NS = res.exec_time_ns
    LAST_TRACE = getattr(res, "instructions_and_trace", None)

    out = np.empty((B, N, D), dtype=np.float32)
    for b in range(B):
        out[b] = res.results[2 * b]["out"] + res.results[2 * b + 1]["out"]
    return out
